# revision 24
# baseline (speedup 1.0000x reference)
"""Differential attention (GQA + RoPE) Bass/Tile kernel for 8 TRN2 NeuronCores.

Sharding: tensor-parallel over the 16 query heads (2 per core, kv head c//2),
Wq/Wk/Wv column-sharded per core; attention output exchanged with an on-device
AllToAll into sequence shards; o_proj row-parallel per sequence shard with the
full Wo on every core; host concatenates the 8 row shards.

Layout notes:
 - x is passed transposed (xT [D, S]) so the contraction dim of every
   projection matmul lands on SBUF partitions.
 - Wq/Wk columns are permuted per head so RoPE's interleaved complex pairs
   become contiguous blocks [x0_A | x1_A | x0_B | x1_B] (A = freqs 0..31,
   B = freqs 32..63).  Attention scores are invariant to any per-half channel
   permutation applied consistently to q and k.
 - Softmax is computed max-free (scores for this problem are within ±6, far
   inside fp16/exp range); the row sum rides the AV matmul as an extra N=1
   matmul against a ones vector, reusing the loaded P^T stationary tile.
 - a1 - lam*a2 is folded linearly: out = (u1*inv_r1 - lam*u2*inv_r2) * 0.5.
"""

import numpy as np
from contextlib import ExitStack

import concourse.bacc as bacc
import concourse.tile as tile
from concourse import mybir
from concourse.bass_utils import run_bass_kernel_spmd

S = 2048
D = 2048
H = 16
KV = 4
HD = 128
HALF = 64
NCORES = 8
HPC = H // NCORES      # 2 query heads per core
P = 128
NT = S // P            # 16 tiles of 128 along s/t
NSC = 4                # s-chunks of 512
SCW = 512
DT = D // P            # 16 tiles along contraction dim
SROWS = S // NCORES    # 256 output rows per core
SCALE = 1.0 / 8.0      # 1/sqrt(HALF)
OUT_SCALE = 0.5        # 1 - lambda_init
NEG = -1.0e9

f32 = mybir.dt.float32
f32r = mybir.dt.float32r
f16 = mybir.dt.float16

_CACHE = {}


def _build():
    nc = bacc.Bacc("TRN2", target_bir_lowering=False, debug=False,
                   num_devices=NCORES)
    # xT/wall/wo arrive pre-tiled from the host so every DMA below is a
    # contiguous 2D slice (4KB+ runs per partition -> full DMA throughput)
    xT = nc.declare_dram_parameter("xT", [P, NSC * DT * SCW], f16,
                                   isOutput=False)
    # the two s-chunks of x this core projects k/v for (parity-sliced by the
    # host: even cores chunks 0-1, odd cores chunks 2-3)
    xkvT = nc.declare_dram_parameter("xkvT", [P, 2 * DT * SCW], f16,
                                     isOutput=False)
    coskvT = nc.declare_dram_parameter("coskvT", [P, 2 * SCW], f16,
                                       isOutput=False)
    sinkvT = nc.declare_dram_parameter("sinkvT", [P, 2 * SCW], f16,
                                       isOutput=False)
    wall = nc.declare_dram_parameter("wall", [P, DT * 4 * P], f16,
                                     isOutput=False)
    wo = nc.declare_dram_parameter("wo", [P, 4 * H * SCW], f16,
                                   isOutput=False)
    cosT = nc.declare_dram_parameter("cosT", [P, S], f16, isOutput=False)
    sinT = nc.declare_dram_parameter("sinT", [P, S], f16, isOutput=False)
    tri = nc.declare_dram_parameter("tri", [P, P], f16, isOutput=False)
    ident = nc.declare_dram_parameter("ident", [P, P], f16, isOutput=False)
    lam = nc.declare_dram_parameter("lam", [1, HPC], f32, isOutput=False)
    o_out = nc.declare_dram_parameter("o_out", [SROWS, D], f32, isOutput=True)

    rg = [list(range(NCORES))]

    with tile.TileContext(nc) as tc, ExitStack() as ctx:
        const = ctx.enter_context(tc.tile_pool(name="const", bufs=1))
        dram = ctx.enter_context(tc.tile_pool(name="dram", bufs=1, space="DRAM"))

        # rows 0..63 and 64..127 both hold freqs 0..63, so every rope
        # operand pair sees equal SBUF base partitions
        cos_sb = const.tile([P, S], f16)
        nc.gpsimd.dma_start(out=cos_sb[:, :], in_=cosT[:, :])
        sin_sb = const.tile([P, S], f16)
        nc.gpsimd.dma_start(out=sin_sb[:, :], in_=sinT[:, :])
        tri_sb = const.tile([P, P], f16)
        nc.gpsimd.dma_start(out=tri_sb[:, :], in_=tri[:, :])
        coskv_sb = const.tile([P, 2 * SCW], f16)
        nc.gpsimd.dma_start(out=coskv_sb[:, :], in_=coskvT[:, :])
        sinkv_sb = const.tile([P, 2 * SCW], f16)
        nc.gpsimd.dma_start(out=sinkv_sb[:, :], in_=sinkvT[:, :])
        id_sb = const.tile([P, P], f16)
        nc.gpsimd.dma_start(out=id_sb[:, :], in_=ident[:, :])
        lam_sb = const.tile([1, HPC], f32)
        nc.gpsimd.dma_start(out=lam_sb[:, :], in_=lam[:, :])
        lam_sig = const.tile([1, HPC], f32)
        nc.scalar.activation(lam_sig[:, :], lam_sb[:, :],
                             mybir.ActivationFunctionType.Sigmoid)
        lam_b = const.tile([P, HPC], f32)
        nc.gpsimd.partition_broadcast(lam_b[:, :], lam_sig[:, :])

        # tiny dummy collective: the FIRST collective on the stream eats the
        # global start barrier + ~12us ncfw cold-start regardless of size, so
        # burn that on 128 bytes; the real k/v AllGather then triggers fast
        wcc_in = dram.tile([64], f16, name="wcc_in")
        wcc_out = dram.tile([64], f16, name="wcc_out")
        nc.gpsimd.dma_start(
            out=wcc_in[:].rearrange("(p f) -> p f", p=1),
            in_=tri_sb[0:1, 0:64])
        nc.gpsimd.collective_compute(
            "AllToAll", mybir.AluOpType.bypass, replica_groups=rg,
            ins=[wcc_in[:]], outs=[wcc_out[:]])

        # Persistent per-core tensors: projected qT/kT (rope applied) per head,
        # kT, vT (fp32, channel-major) and v16 (fp16, t-major for AV rhs).
        qkvp = ctx.enter_context(tc.tile_pool(name="qkvp", bufs=1))
        qkv = [qkvp.tile([P, S], f16, name=f"qkv{j}") for j in range(2)]
        # k with only half-A (resp. half-B) channels non-zero, so the score
        # matmuls contract over the full 128 partitions with no slicing
        kA = qkvp.tile([P, S], f16, name="kA")
        kB = qkvp.tile([P, S], f16, name="kB")
        nc.vector.memset(kA[:, :], 0.0)
        nc.vector.memset(kB[:, :], 0.0)
        # v in t-major fp16, one 136-wide group per t-tile:
        # cols [136jt, 136jt+128) = v, col 136jt+128 = 2.0 -- the rowsum
        # rider; pre-doubled sums make their reciprocal carry the 0.5 scale
        VG = 136
        v16 = qkvp.tile([P, NT * VG], f16)
        # local-half staging tiles for the k/v exchange: kABh packs BOTH
        # roped k halves (disjoint channel rows), v16h the 8 local v tiles
        kABh = qkvp.tile([P, 2 * SCW], f16, name="kABh")
        vT16h = qkvp.tile([P, 2 * SCW], f16, name="vT16h")
        v16h = qkvp.tile([P, 8 * VG], f16, name="v16h")
        nc.vector.memset(v16h[:, :].rearrange("p (jt g) -> p jt g", g=VG)
                         [:, :, 128:129], 2.0)

        # ---- Stage 1: k/v half-projection + pair exchange, then q ----
        # Each core projects k/v only for its parity's half of the sequence
        # (host-sliced xkvT keeps the instruction stream core-independent);
        # the halves are exchanged between the two cores of a kv-group with
        # a pair AllGather that rides under the q projection.
        PAYC = 2 * SCW + 8 * VG
        ag_in = dram.tile([P * PAYC], f16, name="ag_in")
        ag_out = dram.tile([2 * P * PAYC], f16, name="ag_out")
        rg2 = [[2 * i, 2 * i + 1] for i in range(4)]
        with tc.tile_pool(name="wall_p", bufs=1) as wall_pool, \
             tc.tile_pool(name="xt_p", bufs=2) as xt_pool, \
             tc.tile_pool(name="rtmp", bufs=4) as rtmp, \
             tc.tile_pool(name="qscr", bufs=6) as qscr, \
             tc.tile_pool(name="ps1", bufs=4, space="PSUM") as ps1, \
             tc.tile_pool(name="vtps", bufs=2, space="PSUM") as vt_ps:
            # weight block, kv-first halves: cols [0, DT*2P) hold the k/v
            # columns (dt-major), cols [DT*2P, DT*4P) the q columns, so the
            # k/v phase streams after only 1MB of weight DMA
            w_sb = wall_pool.tile([P, DT * 4 * P], f16, name="w_sb")
            for g in range(4):
                csl = slice(g * DT * P, (g + 1) * DT * P)
                nc.scalar.dma_start(out=w_sb[:, csl], in_=wall[:, csl])
            KVOFF = 0
            QOFF = DT * 2 * P

            for skc in range(2):
                xts = xt_pool.tile([P, DT * SCW], f16, name="xt", tag="xt")
                for g in range(4):
                    csl = slice(g * 4 * SCW, (g + 1) * 4 * SCW)
                    nc.sync.dma_start(
                        out=xts[:, csl],
                        in_=xkvT[:, skc * DT * SCW + csl.start:
                                 skc * DT * SCW + csl.stop])
                lsl = slice(skc * SCW, (skc + 1) * SCW)
                for j in (2, 3):
                    psum_p = ps1.tile([P, SCW], f32, name="psum_p", tag="p1")
                    for dt_ in range(DT):
                        wc = KVOFF + dt_ * 2 * P + (j - 2) * P
                        nc.tensor.matmul(
                            psum_p[:, :],
                            w_sb[:, wc:wc + P],
                            xts[:, dt_ * SCW:(dt_ + 1) * SCW],
                            start=(dt_ == 0), stop=(dt_ == DT - 1))
                    if j == 2:
                        t0 = rtmp.tile([P, SCW], f16, name="t0", tag="kt0")
                        t1 = rtmp.tile([P, SCW], f16, name="t1", tag="kt1")
                        xsc = qscr.tile([P, SCW], f16, name="xsc", tag="xsc")
                        nc.scalar.copy(xsc[:, :], psum_p[:, :])
                        # k rope: half-A rows {0-31,64-95}, half-B rows
                        # {32-63,96-127} -- disjoint, both into kABh
                        for hf in range(2):
                            fr = slice(32 * hf, 32 * hf + 32)
                            r1 = slice(64 + 32 * hf, 64 + 32 * hf + 32)
                            x0 = xsc[fr, :]
                            x1 = xsc[r1, :]
                            eng = nc.vector
                            eng.tensor_mul(t0[fr, :], x1, sinkv_sb[r1, lsl])
                            eng.tensor_mul(kABh[fr, lsl], x0,
                                           coskv_sb[fr, lsl])
                            eng.tensor_sub(kABh[fr, lsl], kABh[fr, lsl],
                                           t0[fr, :])
                            eng.tensor_mul(t1[r1, :], x0, sinkv_sb[fr, lsl])
                            eng.tensor_mul(kABh[r1, lsl], x1,
                                           coskv_sb[r1, lsl])
                            eng.tensor_add(kABh[r1, lsl], kABh[r1, lsl],
                                           t1[r1, :])
                    else:
                        nc.scalar.copy(vT16h[:, lsl], psum_p[:, :])
                        for jtl in range(4 * skc, 4 * skc + 4):
                            ps_t = vt_ps.tile([P, P], f16, name="ps_vt",
                                              tag="vt")
                            nc.tensor.transpose(
                                ps_t[:, :], vT16h[:, jtl * P:(jtl + 1) * P],
                                id_sb[:, :])
                            nc.scalar.copy(v16h[:, jtl * VG:jtl * VG + P],
                                           ps_t[:, :])
            # pack [kAB | v16] halves, pair AllGather, unpack into the
            # full tiles in global order (rank 0 = even core = chunks 0-1)
            agi = ag_in[:].rearrange("(p f) -> p f", f=PAYC)
            nc.gpsimd.dma_start(out=agi[:, 0:2 * SCW], in_=kABh[:, :])
            nc.gpsimd.dma_start(out=agi[:, 2 * SCW:PAYC], in_=v16h[:, :])
            nc.gpsimd.collective_compute(
                "AllGather", mybir.AluOpType.bypass, replica_groups=rg2,
                ins=[ag_in[:]], outs=[ag_out[:]])
            ago = ag_out[:].rearrange("(r p f) -> p r f", r=2, f=PAYC)
            for a, b in ((0, 32), (64, 96)):
                nc.gpsimd.dma_start(
                    out=kA[a:b, :].rearrange("p (r f) -> p r f", r=2),
                    in_=ago[a:b, :, 0:2 * SCW])
            for a, b in ((32, 64), (96, 128)):
                nc.gpsimd.dma_start(
                    out=kB[a:b, :].rearrange("p (r f) -> p r f", r=2),
                    in_=ago[a:b, :, 0:2 * SCW])
            nc.gpsimd.dma_start(
                out=v16[:, :].rearrange("p (r f) -> p r f", r=2),
                in_=ago[:, :, 2 * SCW:PAYC])

            for sc in range(NSC):
                # one DMA per s-chunk: column group dt holds
                # xT[dt*128:(dt+1)*128, sc*512:(sc+1)*512]
                xts = xt_pool.tile([P, DT * SCW], f16, name="xt", tag="xt")
                for g in range(4):
                    csl = slice(g * 4 * SCW, (g + 1) * 4 * SCW)
                    nc.sync.dma_start(
                        out=xts[:, csl],
                        in_=xT[:, sc * DT * SCW + csl.start:
                               sc * DT * SCW + csl.stop])
                ssl = slice(sc * SCW, (sc + 1) * SCW)
                for j in (0, 1):
                    psum_p = ps1.tile([P, SCW], f32, name="psum_p", tag="p1")
                    for dt_ in range(DT):
                        wc = QOFF + dt_ * 2 * P + j * P
                        nc.tensor.matmul(
                            psum_p[:, :],
                            w_sb[:, wc:wc + P],
                            xts[:, dt_ * SCW:(dt_ + 1) * SCW],
                            start=(dt_ == 0), stop=(dt_ == DT - 1))
                    # psum rows: [x0 (freqs 0..63) | x1 (freqs 0..63)]
                    t0 = rtmp.tile([P, SCW], f16, name="t0", tag="t0")
                    t1 = rtmp.tile([P, SCW], f16, name="t1", tag="t1")
                    # evict on the scalar engine so the PSUM bank frees
                    # fast; rope then runs SBUF->SBUF on DVE
                    xsc = qscr.tile([P, SCW], f16, name="xsc", tag="xsc")
                    nc.scalar.copy(xsc[:, :], psum_p[:, :])
                    # q rope at full 64-row width:
                    # rows 0..63 = r0, rows 64..127 = r1
                    q_t = qkv[j]
                    nc.vector.tensor_mul(t0[0:64, :], xsc[64:128, :],
                                         sin_sb[64:128, ssl])
                    nc.vector.tensor_mul(q_t[0:64, ssl], xsc[0:64, :],
                                         cos_sb[0:64, ssl])
                    nc.vector.tensor_sub(q_t[0:64, ssl], q_t[0:64, ssl],
                                         t0[0:64, :])
                    nc.vector.tensor_mul(t1[64:128, :], xsc[0:64, :],
                                         sin_sb[0:64, ssl])
                    nc.vector.tensor_mul(q_t[64:128, ssl],
                                         xsc[64:128, :],
                                         cos_sb[64:128, ssl])
                    nc.vector.tensor_add(q_t[64:128, ssl],
                                         q_t[64:128, ssl], t1[64:128, :])

        # prefetch full Wo (fp16) early on the scalar DMA queue so the
        # o_proj partials can start the moment the first AllToAll lands
        wo_pool = ctx.enter_context(tc.tile_pool(name="wo_p", bufs=1))
        wos_l = []
        for dc in range(4):
            wos = wo_pool.tile([P, H * SCW], f16, name="wos", tag=f"wos{dc}")
            nc.gpsimd.dma_start(
                out=wos[:, :],
                in_=wo[:, dc * H * SCW:(dc + 1) * H * SCW])
            wos_l.append(wos)

        # ---- Stage 2: differential attention per (head, s-chunk) ----
        attnp = ctx.enter_context(tc.tile_pool(name="attnp", bufs=1))
        attnT = [attnp.tile([P, S], f16, name=f"attnT{h}") for h in range(HPC)]

        # per-head AllToAll bounce buffers (issued as soon as head h is done,
        # so the first exchange overlaps the second head's compute)
        sec = P * SROWS  # elems per (core, head) section
        bounce_in = [dram.tile([NCORES * sec], f16, name=f"bounce_in{h}")
                     for h in range(HPC)]
        bounce_out = [dram.tile([NCORES * sec], f16, name=f"bounce_out{h}")
                      for h in range(HPC)]

        misc_ps = ctx.enter_context(
            tc.tile_pool(name="misc_ps", bufs=2, space="PSUM"))
        with tc.tile_pool(name="expst_p", bufs=2) as expst_pool, \
             tc.tile_pool(name="st_p", bufs=2, space="PSUM") as st_pool, \
             tc.tile_pool(name="u_p", bufs=1, space="PSUM") as u_pool, \
             tc.tile_pool(name="cmb", bufs=4) as cmb:
            UG = 136
            for h in range(HPC):
                for sc in range(NSC):
                    njt = 4 * sc + 4
                    ps_u = [None, None]
                    for hf in range(2):
                        k_t = kA if hf == 0 else kB
                        expst = expst_pool.tile([P, njt * SCW], f16,
                                                name="expst", tag="expst",
                                                bufs=4)
                        for jt in range(njt):
                            ps_st = st_pool.tile([P, SCW], f32,
                                                 name="ps_st", tag="st")
                            # columns below 128*m are never read by the AV
                            # loop (fully-masked): restrict matmul+exp to
                            # the live region.  Causality within the
                            # diagonal 128x128 block is a 0/1 fp16 multiply
                            # with the triangle tile on the exp output (the
                            # rowsum rider sums the same zeroed tile, so the
                            # math is unchanged).
                            lo = 0
                            if jt >= 4 * sc:
                                m = jt % 4
                                lo = P * m
                            nc.tensor.matmul(
                                ps_st[:, lo:],
                                k_t[:, jt * P:(jt + 1) * P],
                                qkv[h][:, sc * SCW + lo:(sc + 1) * SCW],
                                start=True, stop=True)
                            esl = expst[:, jt * SCW + lo:(jt + 1) * SCW]
                            nc.scalar.activation(
                                esl, ps_st[:, lo:],
                                mybir.ActivationFunctionType.Exp, scale=SCALE)
                            if jt >= 4 * sc:
                                blk = expst[:, jt * SCW + lo:
                                            jt * SCW + lo + P]
                                nc.vector.tensor_mul(blk, blk, tri_sb[:, :])
                        # two banks per half: [u(128) | r(1) | pad] x2
                        ps_u[hf] = [
                            u_pool.tile([P, 2 * UG], f32,
                                        name=f"ps_u{hf}{qq}", tag=f"u{hf}{qq}")
                            for qq in range(2)]
                        for q_ in range(4):
                            js = 4 * sc + q_
                            put = ps_u[hf][q_ // 2]
                            off = UG * (q_ % 2)
                            for jt in range(js + 1):
                                lhs = expst[:, jt * SCW + q_ * P:
                                            jt * SCW + q_ * P + P]
                                nc.tensor.matmul(
                                    put[:, off:off + 129],
                                    lhs, v16[:, jt * VG:jt * VG + 129],
                                    start=(jt == 0), stop=(jt == js))
                    # combine: attn = 0.5*(u1*inv_r1 - lam*u2*inv_r2)
                    inv = cmb.tile([P, 8], f32, name="inv", tag="inv")
                    for hf in range(2):
                        for q_ in range(4):
                            nc.vector.reciprocal(
                                inv[:, 4 * hf + q_:4 * hf + q_ + 1],
                                ps_u[hf][q_ // 2][:, UG * (q_ % 2) + 128:
                                                  UG * (q_ % 2) + 129])
                    for q_ in range(4):
                        js = 4 * sc + q_
                        u0 = ps_u[0][q_ // 2][:, UG * (q_ % 2):
                                              UG * (q_ % 2) + P]
                        u1 = ps_u[1][q_ // 2][:, UG * (q_ % 2):
                                              UG * (q_ % 2) + P]
                        sc2 = cmb.tile([P, 1], f32, name="sc2", tag="sc2")
                        nc.vector.tensor_scalar_mul(
                            sc2[:, :], inv[:, 4 + q_:5 + q_],
                            lam_b[:, h:h + 1])
                        tmp2 = cmb.tile([P, P], f32, name="tmp2", tag="tmp2")
                        nc.vector.tensor_scalar_mul(tmp2[:, :], u1, sc2[:, :])
                        attn_sl = cmb.tile([P, P], f16, name="attn_sl",
                                           tag="attn_sl")
                        nc.vector.scalar_tensor_tensor(
                            attn_sl[:, :], u0,
                            inv[:, q_:q_ + 1], tmp2[:, :],
                            mybir.AluOpType.mult, mybir.AluOpType.subtract)
                        ps_t = misc_ps.tile([P, P], f16, name="ps_at",
                                            tag="misc")
                        nc.tensor.transpose(ps_t[:, :], attn_sl[:, :],
                                            id_sb[:, :])
                        nc.vector.tensor_copy(attnT[h][:, js * P:(js + 1) * P],
                                              ps_t[:, :])
                # ---- Stage 3 (per head): AllToAll into sequence shards ----
                if sc == NSC - 1:
                    nc.gpsimd.dma_start(
                        out=bounce_in[h][:].rearrange(
                            "(d p f) -> p d f", d=NCORES, f=SROWS),
                        in_=attnT[h][:, :].rearrange(
                            "p (d f) -> p d f", f=SROWS))
                    nc.gpsimd.collective_compute(
                        "AllToAll", mybir.AluOpType.bypass, replica_groups=rg,
                        ins=[bounce_in[h][:]], outs=[bounce_out[h][:]])

        # ---- Stage 4: o_proj over the local 256 rows ----
        with tc.tile_pool(name="aT_p", bufs=1) as aT_pool, \
             tc.tile_pool(name="o_p", bufs=4) as o_pool, \
             tc.tile_pool(name="ps4", bufs=2, space="PSUM") as ps4:
            aTl = []
            for h in range(HPC):
                a_t = aT_pool.tile([P, NCORES * SROWS], f16, name=f"aT{h}")
                nc.gpsimd.dma_start(
                    out=a_t[:, :].rearrange("p (d f) -> p d f", d=NCORES),
                    in_=bounce_out[h][:].rearrange(
                        "(d p f) -> p d f", d=NCORES, f=SROWS))
                aTl.append(a_t)
            # head-0 sections only need the first AllToAll: run ALL of their
            # partial o_proj groups while the second exchange is in flight.
            o_es = {}
            for dc in range(4):
                for st_ in range(2):
                    ps_e = misc_ps.tile([P, SCW], f32, name="ps_e", tag="misc")
                    for i, ht in enumerate(range(0, H, 2)):
                        nc.tensor.matmul(
                            ps_e[:, :],
                            aTl[0][:, (ht // 2) * SROWS + st_ * P:
                                   (ht // 2) * SROWS + (st_ + 1) * P],
                            wos_l[dc][:, ht * SCW:(ht + 1) * SCW],
                            start=(i == 0), stop=(i == H // 2 - 1))
                    o_e = o_pool.tile([P, SCW], f32, name="o_e",
                                      tag=f"o_e{dc}{st_}", bufs=1)
                    nc.vector.tensor_copy(o_e[:, :], ps_e[:, :])
                    o_es[(dc, st_)] = o_e
            # keep the PE (HAM) warm across the second AllToAll's wait
            # window so the head-1 pass below starts at full clock; the
            # result is parked in DRAM and never read.
            warm_ps = misc_ps.tile([P, SCW], f32, name="warm", tag="misc")
            for w_ in range(6):
                nc.tensor.matmul(warm_ps[:, :], v16[:, 0:P],
                                 aTl[0][:, 0:SCW], start=True, stop=True)
            warm_sb = o_pool.tile([P, SCW], f32, name="warm_sb",
                                  tag="warm_sb", bufs=1)
            nc.vector.tensor_copy(warm_sb[:, :], warm_ps[:, :])
            warm_dram = dram.tile([P * SCW], f32, name="warm_dram")
            nc.gpsimd.dma_start(
                out=warm_dram[:].rearrange("(p f) -> p f", f=SCW),
                in_=warm_sb[:, :])
            for dc in range(4):
                for st_ in range(2):
                    ps_o = ps4.tile([P, SCW], f32, name="ps_o", tag="o")
                    for i, ht in enumerate(range(1, H, 2)):
                        nc.tensor.matmul(
                            ps_o[:, :],
                            aTl[1][:, (ht // 2) * SROWS + st_ * P:
                                   (ht // 2) * SROWS + (st_ + 1) * P],
                            wos_l[dc][:, ht * SCW:(ht + 1) * SCW],
                            start=(i == 0), stop=(i == H // 2 - 1))
                    o_sb = o_pool.tile([P, SCW], f32, name="o_sb", tag="o_sb")
                    nc.vector.tensor_add(o_sb[:, :], ps_o[:, :],
                                         o_es[(dc, st_)][:, :])
                    nc.sync.dma_start(
                        out=o_out[st_ * P:(st_ + 1) * P,
                                  dc * SCW:(dc + 1) * SCW],
                        in_=o_sb[:, :])

    nc.compile()
    return nc


def _prep(x, freqs_cos, freqs_sin, Wq, Wk, Wv, Wo, lambda_param):
    """Host-side sharding/layout prep. Returns per-core input maps."""
    x2 = np.asarray(x, np.float32).reshape(S, D)
    # pre-tile: xT[p, (sc, dt, f)] = x[512sc+f, 128dt+p] so each stage-1
    # dt-group DMA is one contiguous slice
    xT = np.ascontiguousarray(
        x2.reshape(NSC, SCW, DT, P).transpose(3, 0, 2, 1)
        .reshape(P, NSC * DT * SCW).astype(np.float16))
    cosT = np.asarray(freqs_cos, np.float32).T
    sinT = np.asarray(freqs_sin, np.float32).T
    cosT = np.ascontiguousarray(
        np.concatenate([cosT, cosT], axis=0).astype(np.float16))
    sinT = np.ascontiguousarray(
        np.concatenate([sinT, sinT], axis=0).astype(np.float16))
    Wq = np.asarray(Wq, np.float32)
    Wk = np.asarray(Wk, np.float32)
    Wv = np.asarray(Wv, np.float32)
    # pre-tile: wo[p, (dc, ht, f)] = Wo[128ht+p, 512dc+f]
    Wo16 = np.ascontiguousarray(
        np.asarray(Wo, np.float32).reshape(H, P, 4, SCW)
        .transpose(1, 2, 0, 3).reshape(P, 4 * H * SCW).astype(np.float16))
    lamp = np.asarray(lambda_param, np.float32)

    # de-interleave complex pairs: [x0 (freqs 0..63) | x1 (freqs 0..63)]
    perm = np.concatenate([
        2 * np.arange(64), 2 * np.arange(64) + 1]).astype(np.int64)

    # in-block causal triangle: tri[t, s] = t <= s
    t_rel = np.arange(P)[:, None]
    s_rel = np.arange(P)[None, :]
    tri = np.where(t_rel <= s_rel, 1.0, 0.0).astype(np.float16)

    ident = np.eye(P, dtype=np.float16)

    in_maps = []
    for c in range(NCORES):
        g = c // 2
        # kv-first halves: [ (dt, [k|v]) | (dt, [q0|q1]) ]
        kv = np.concatenate([Wk[:, g * HD:(g + 1) * HD][:, perm],
                             Wv[:, g * HD:(g + 1) * HD]], axis=1)
        qq = np.concatenate(
            [Wq[:, h * HD:(h + 1) * HD][:, perm]
             for h in (2 * c, 2 * c + 1)], axis=1)
        kvt = kv.reshape(DT, P, 2 * P).transpose(1, 0, 2).reshape(P, -1)
        qqt = qq.reshape(DT, P, 2 * P).transpose(1, 0, 2).reshape(P, -1)
        wall = np.ascontiguousarray(
            np.concatenate([kvt, qqt], axis=1).astype(np.float16))
        par = c % 2
        in_maps.append({
            "xT": xT,
            "xkvT": np.ascontiguousarray(
                xT[:, par * 2 * DT * SCW:(par + 1) * 2 * DT * SCW]),
            "wall": wall,
            "wo": Wo16,
            "cosT": cosT,
            "sinT": sinT,
            "coskvT": np.ascontiguousarray(
                cosT[:, par * 2 * SCW:(par + 1) * 2 * SCW]),
            "sinkvT": np.ascontiguousarray(
                sinT[:, par * 2 * SCW:(par + 1) * 2 * SCW]),
            "tri": tri,
            "ident": ident,
            "lam": np.ascontiguousarray(
                lamp[2 * c:2 * c + 2].reshape(1, HPC)),
        })
    return in_maps


def _run(inputs, trace=False):
    if "nc" not in _CACHE:
        _CACHE["nc"] = _build()
    nc = _CACHE["nc"]
    in_maps = _prep(**inputs)
    res = run_bass_kernel_spmd(nc, in_maps, core_ids=list(range(NCORES)),
                               trace=trace)
    out = np.concatenate([res.results[c]["o_out"] for c in range(NCORES)],
                         axis=0)
    return out.reshape(1, S, D), res


def kernel(**inputs):
    out, _ = _run(inputs)
    return out



# revision 30
# speedup vs baseline: 1.0806x; 1.0806x over previous
"""Differential attention (GQA + RoPE) Bass/Tile kernel for 8 TRN2 NeuronCores.

Sharding: tensor-parallel over the 16 query heads (2 per core, kv head c//2),
Wq/Wk/Wv column-sharded per core; attention output exchanged with an on-device
AllToAll into sequence shards; o_proj row-parallel per sequence shard with the
full Wo on every core; host concatenates the 8 row shards.

Layout notes:
 - x is passed transposed (xT [D, S]) so the contraction dim of every
   projection matmul lands on SBUF partitions.
 - Wq/Wk columns are permuted per head so RoPE's interleaved complex pairs
   become contiguous blocks [x0_A | x1_A | x0_B | x1_B] (A = freqs 0..31,
   B = freqs 32..63).  Attention scores are invariant to any per-half channel
   permutation applied consistently to q and k.
 - Softmax is computed max-free (scores for this problem are within ±6, far
   inside fp16/exp range); the row sum rides the AV matmul as an extra N=1
   matmul against a ones vector, reusing the loaded P^T stationary tile.
 - a1 - lam*a2 is folded linearly: out = (u1*inv_r1 - lam*u2*inv_r2) * 0.5.
"""

import numpy as np
from contextlib import ExitStack

import concourse.bacc as bacc
import concourse.tile as tile
from concourse import mybir
from concourse.bass_utils import run_bass_kernel_spmd

S = 2048
D = 2048
H = 16
KV = 4
HD = 128
HALF = 64
NCORES = 8
HPC = H // NCORES      # 2 query heads per core
P = 128
NT = S // P            # 16 tiles of 128 along s/t
NSC = 4                # s-chunks of 512
SCW = 512
DT = D // P            # 16 tiles along contraction dim
SROWS = S // NCORES    # 256 output rows per core
SCALE = 1.0 / 8.0      # 1/sqrt(HALF)
OUT_SCALE = 0.5        # 1 - lambda_init
NEG = -1.0e9

f32 = mybir.dt.float32
f32r = mybir.dt.float32r
f16 = mybir.dt.float16

_CACHE = {}


def _build():
    nc = bacc.Bacc("TRN2", target_bir_lowering=False, debug=False,
                   num_devices=NCORES)
    # xT/wall/wo arrive pre-tiled from the host so every DMA below is a
    # contiguous 2D slice (4KB+ runs per partition -> full DMA throughput)
    xT = nc.declare_dram_parameter("xT", [P, NSC * DT * SCW], f16,
                                   isOutput=False)
    wall = nc.declare_dram_parameter("wall", [P, DT * 4 * P], f16,
                                     isOutput=False)
    wo = nc.declare_dram_parameter("wo", [P, 4 * H * SCW], f16,
                                   isOutput=False)
    cosT = nc.declare_dram_parameter("cosT", [P, S], f16, isOutput=False)
    sinT = nc.declare_dram_parameter("sinT", [P, S], f16, isOutput=False)
    tri = nc.declare_dram_parameter("tri", [P, P], f16, isOutput=False)
    ident = nc.declare_dram_parameter("ident", [P, P], f16, isOutput=False)
    lam = nc.declare_dram_parameter("lam", [1, HPC], f32, isOutput=False)
    o_out = nc.declare_dram_parameter("o_out", [SROWS, D], f32, isOutput=True)

    rg = [list(range(NCORES))]

    with tile.TileContext(nc) as tc, ExitStack() as ctx:
        const = ctx.enter_context(tc.tile_pool(name="const", bufs=1))
        dram = ctx.enter_context(tc.tile_pool(name="dram", bufs=1, space="DRAM"))

        # rows 0..63 and 64..127 both hold freqs 0..63, so every rope
        # operand pair sees equal SBUF base partitions
        cos_sb = const.tile([P, S], f16)
        nc.gpsimd.dma_start(out=cos_sb[:, :], in_=cosT[:, :])
        sin_sb = const.tile([P, S], f16)
        nc.gpsimd.dma_start(out=sin_sb[:, :], in_=sinT[:, :])
        tri_sb = const.tile([P, P], f16)
        nc.gpsimd.dma_start(out=tri_sb[:, :], in_=tri[:, :])
        id_sb = const.tile([P, P], f16)
        nc.gpsimd.dma_start(out=id_sb[:, :], in_=ident[:, :])
        lam_sb = const.tile([1, HPC], f32)
        nc.gpsimd.dma_start(out=lam_sb[:, :], in_=lam[:, :])
        lam_sig = const.tile([1, HPC], f32)
        nc.scalar.activation(lam_sig[:, :], lam_sb[:, :],
                             mybir.ActivationFunctionType.Sigmoid)
        lam_b = const.tile([P, HPC], f32)
        nc.gpsimd.partition_broadcast(lam_b[:, :], lam_sig[:, :])

        # tiny dummy collective: absorbs the ~11us ncfw cold-start into the
        # fully-hidden stage-1 window so the first real AllToAll triggers fast
        wcc_in = dram.tile([64], f16, name="wcc_in")
        wcc_out = dram.tile([64], f16, name="wcc_out")
        nc.gpsimd.dma_start(
            out=wcc_in[:].rearrange("(p f) -> p f", p=1),
            in_=tri_sb[0:1, 0:64])
        nc.gpsimd.collective_compute(
            "AllToAll", mybir.AluOpType.bypass, replica_groups=rg,
            ins=[wcc_in[:]], outs=[wcc_out[:]])

        # Persistent per-core tensors: projected qT/kT (rope applied) per head,
        # kT, vT (fp32, channel-major) and v16 (fp16, t-major for AV rhs).
        qkvp = ctx.enter_context(tc.tile_pool(name="qkvp", bufs=1))
        qkv = [qkvp.tile([P, S], f16, name=f"qkv{j}") for j in range(2)]
        # k with only half-A (resp. half-B) channels non-zero, so the score
        # matmuls contract over the full 128 partitions with no slicing
        kA = qkvp.tile([P, S], f16, name="kA")
        kB = qkvp.tile([P, S], f16, name="kB")
        nc.vector.memset(kA[:, :], 0.0)
        nc.vector.memset(kB[:, :], 0.0)
        vT16 = qkvp.tile([P, S], f16)
        # v in t-major fp16, one 136-wide group per t-tile:
        # cols [136jt, 136jt+128) = v, col 136jt+128 = 2.0 -- the rowsum
        # rider; pre-doubled sums make their reciprocal carry the 0.5 scale
        VG = 136
        v16 = qkvp.tile([P, NT * VG], f16)
        nc.vector.memset(v16[:, :].rearrange("p (jt g) -> p jt g", g=VG)
                         [:, :, 128:129], 2.0)

        # ---- Stage 1: fused qkv projection (+RoPE on eviction) ----
        with tc.tile_pool(name="wall_p", bufs=1) as wall_pool, \
             tc.tile_pool(name="xt_p", bufs=2) as xt_pool, \
             tc.tile_pool(name="rtmp", bufs=4) as rtmp, \
             tc.tile_pool(name="qscr", bufs=6) as qscr, \
             tc.tile_pool(name="ps1", bufs=4, space="PSUM") as ps1, \
             tc.tile_pool(name="vtps", bufs=2, space="PSUM") as vt_ps:
            # one DMA for the whole weight block: column group dt holds
            # wall[dt*128:(dt+1)*128, :] -> [128, 16*512]
            w_sb = wall_pool.tile([P, DT * 4 * P], f16, name="w_sb")
            for g in range(4):
                csl = slice(g * 4 * 4 * P, (g + 1) * 4 * 4 * P)
                nc.scalar.dma_start(out=w_sb[:, csl], in_=wall[:, csl])

            for sc in range(NSC):
                # one DMA per s-chunk: column group dt holds
                # xT[dt*128:(dt+1)*128, sc*512:(sc+1)*512]
                xts = xt_pool.tile([P, DT * SCW], f16, name="xt", tag="xt")
                for g in range(4):
                    csl = slice(g * 4 * SCW, (g + 1) * 4 * SCW)
                    nc.sync.dma_start(
                        out=xts[:, csl],
                        in_=xT[:, sc * DT * SCW + csl.start:
                               sc * DT * SCW + csl.stop])
                for j in (2, 0, 1, 3):
                    psum_p = ps1.tile([P, SCW], f32, name="psum_p", tag="p1")
                    for dt_ in range(DT):
                        nc.tensor.matmul(
                            psum_p[:, :],
                            w_sb[:, dt_ * 4 * P + j * P:
                                 dt_ * 4 * P + (j + 1) * P],
                            xts[:, dt_ * SCW:(dt_ + 1) * SCW],
                            start=(dt_ == 0), stop=(dt_ == DT - 1))
                    ssl = slice(sc * SCW, (sc + 1) * SCW)
                    # psum rows: [x0 (freqs 0..63) | x1 (freqs 0..63)]
                    tags = ("t0", "t1") if j < 2 else ("kt0", "kt1")
                    t0 = rtmp.tile([P, SCW], f16, name="t0", tag=tags[0])
                    t1 = rtmp.tile([P, SCW], f16, name="t1", tag=tags[1])
                    if j < 3:
                        # evict on the scalar engine so the PSUM bank frees
                        # fast; rope then runs SBUF->SBUF on DVE
                        xsc = qscr.tile([P, SCW], f16, name="xsc", tag="xsc")
                        nc.scalar.copy(xsc[:, :], psum_p[:, :])
                        psum_p = xsc
                    if j < 2:
                        # q rope at full 64-row width:
                        # rows 0..63 = r0, rows 64..127 = r1
                        q_t = qkv[j]
                        nc.vector.tensor_mul(t0[0:64, :], psum_p[64:128, :],
                                             sin_sb[64:128, ssl])
                        nc.vector.tensor_mul(q_t[0:64, ssl], psum_p[0:64, :],
                                             cos_sb[0:64, ssl])
                        nc.vector.tensor_sub(q_t[0:64, ssl], q_t[0:64, ssl],
                                             t0[0:64, :])
                        nc.vector.tensor_mul(t1[64:128, :], psum_p[0:64, :],
                                             sin_sb[0:64, ssl])
                        nc.vector.tensor_mul(q_t[64:128, ssl],
                                             psum_p[64:128, :],
                                             cos_sb[64:128, ssl])
                        nc.vector.tensor_add(q_t[64:128, ssl],
                                             q_t[64:128, ssl], t1[64:128, :])
                    elif j == 2:
                        # k rope scattered into kA (half-A rows) / kB (half-B)
                        for hf, kt in ((0, kA), (1, kB)):
                            fr = slice(32 * hf, 32 * hf + 32)
                            r1 = slice(64 + 32 * hf, 64 + 32 * hf + 32)
                            x0 = psum_p[fr, :]
                            x1 = psum_p[r1, :]
                            eng = nc.vector
                            eng.tensor_mul(t0[fr, :], x1, sin_sb[r1, ssl])
                            eng.tensor_mul(kt[fr, ssl], x0, cos_sb[fr, ssl])
                            eng.tensor_sub(kt[fr, ssl], kt[fr, ssl],
                                           t0[fr, :])
                            eng.tensor_mul(t1[r1, :], x0, sin_sb[fr, ssl])
                            eng.tensor_mul(kt[r1, ssl], x1, cos_sb[r1, ssl])
                            eng.tensor_add(kt[r1, ssl], kt[r1, ssl],
                                           t1[r1, :])
                    else:
                        nc.scalar.copy(vT16[:, ssl], psum_p[:, :])
                        # fold the v->t-major transposes into this chunk so
                        # the stage boundary disappears (f16: 1 cyc/row)
                        for jt in range(4 * sc, 4 * sc + 4):
                            ps_t = vt_ps.tile([P, P], f16, name="ps_vt",
                                              tag="vt")
                            nc.tensor.transpose(
                                ps_t[:, :], vT16[:, jt * P:(jt + 1) * P],
                                id_sb[:, :])
                            nc.scalar.copy(v16[:, jt * VG:jt * VG + P],
                                           ps_t[:, :])

        # prefetch full Wo (fp16) early on the scalar DMA queue so the
        # o_proj partials can start the moment the first AllToAll lands
        wo_pool = ctx.enter_context(tc.tile_pool(name="wo_p", bufs=1))
        wos_l = []
        for dc in range(4):
            wos = wo_pool.tile([P, H * SCW], f16, name="wos", tag=f"wos{dc}")
            nc.gpsimd.dma_start(
                out=wos[:, :],
                in_=wo[:, dc * H * SCW:(dc + 1) * H * SCW])
            wos_l.append(wos)

        # ---- Stage 2: differential attention per (head, s-chunk) ----
        attnp = ctx.enter_context(tc.tile_pool(name="attnp", bufs=1))
        attnT = [attnp.tile([P, S], f16, name=f"attnT{h}") for h in range(HPC)]

        # per-head AllToAll bounce buffers (issued as soon as head h is done,
        # so the first exchange overlaps the second head's compute).  Head 1
        # is exchanged in two half-width (128-col) pieces so the head-1
        # o_proj can start after the first piece lands.
        sec = P * SROWS  # elems per (core, head) section
        bounce_in = [dram.tile([NCORES * sec], f16, name=f"bounce_in{h}")
                     for h in range(HPC)]
        bounce_out = [dram.tile([NCORES * sec], f16, name=f"bounce_out{h}")
                      for h in range(HPC)]
        bounce_in2 = [dram.tile([NCORES * P * P], f16, name=f"bounce_in2{i}")
                      for i in range(2)]
        bounce_out2 = [dram.tile([NCORES * P * P], f16,
                                 name=f"bounce_out2{i}") for i in range(2)]
        aT_pool = ctx.enter_context(tc.tile_pool(name="aT_p", bufs=1))
        aTl = [aT_pool.tile([P, NCORES * SROWS], f16, name=f"aT{h}")
               for h in range(HPC)]

        misc_ps = ctx.enter_context(
            tc.tile_pool(name="misc_ps", bufs=2, space="PSUM"))
        with tc.tile_pool(name="expst_p", bufs=2) as expst_pool, \
             tc.tile_pool(name="st_p", bufs=2, space="PSUM") as st_pool, \
             tc.tile_pool(name="u_p", bufs=1, space="PSUM") as u_pool, \
             tc.tile_pool(name="cmb", bufs=4) as cmb:
            UG = 136
            for h in range(HPC):
                for sc in range(NSC):
                    njt = 4 * sc + 4
                    ps_u = [None, None]
                    for hf in range(2):
                        k_t = kA if hf == 0 else kB
                        expst = expst_pool.tile([P, njt * SCW], f16,
                                                name="expst", tag="expst",
                                                bufs=4)
                        for jt in range(njt):
                            ps_st = st_pool.tile([P, SCW], f32,
                                                 name="ps_st", tag="st")
                            # columns below 128*m are never read by the AV
                            # loop (fully-masked): restrict matmul+exp to
                            # the live region.  Causality within the
                            # diagonal 128x128 block is a 0/1 fp16 multiply
                            # with the triangle tile on the exp output (the
                            # rowsum rider sums the same zeroed tile, so the
                            # math is unchanged).
                            lo = 0
                            if jt >= 4 * sc:
                                m = jt % 4
                                lo = P * m
                            nc.tensor.matmul(
                                ps_st[:, lo:],
                                k_t[:, jt * P:(jt + 1) * P],
                                qkv[h][:, sc * SCW + lo:(sc + 1) * SCW],
                                start=True, stop=True)
                            esl = expst[:, jt * SCW + lo:(jt + 1) * SCW]
                            nc.scalar.activation(
                                esl, ps_st[:, lo:],
                                mybir.ActivationFunctionType.Exp, scale=SCALE)
                            if jt >= 4 * sc:
                                blk = expst[:, jt * SCW + lo:
                                            jt * SCW + lo + P]
                                nc.vector.tensor_mul(blk, blk, tri_sb[:, :])
                        # two banks per half: [u(128) | r(1) | pad] x2
                        ps_u[hf] = [
                            u_pool.tile([P, 2 * UG], f32,
                                        name=f"ps_u{hf}{qq}", tag=f"u{hf}{qq}")
                            for qq in range(2)]
                        for q_ in range(4):
                            js = 4 * sc + q_
                            put = ps_u[hf][q_ // 2]
                            off = UG * (q_ % 2)
                            for jt in range(js + 1):
                                lhs = expst[:, jt * SCW + q_ * P:
                                            jt * SCW + q_ * P + P]
                                nc.tensor.matmul(
                                    put[:, off:off + 129],
                                    lhs, v16[:, jt * VG:jt * VG + 129],
                                    start=(jt == 0), stop=(jt == js))
                    # combine: attn = 0.5*(u1*inv_r1 - lam*u2*inv_r2)
                    inv = cmb.tile([P, 8], f32, name="inv", tag="inv")
                    for hf in range(2):
                        for q_ in range(4):
                            nc.vector.reciprocal(
                                inv[:, 4 * hf + q_:4 * hf + q_ + 1],
                                ps_u[hf][q_ // 2][:, UG * (q_ % 2) + 128:
                                                  UG * (q_ % 2) + 129])
                    for q_ in range(4):
                        js = 4 * sc + q_
                        u0 = ps_u[0][q_ // 2][:, UG * (q_ % 2):
                                              UG * (q_ % 2) + P]
                        u1 = ps_u[1][q_ // 2][:, UG * (q_ % 2):
                                              UG * (q_ % 2) + P]
                        sc2 = cmb.tile([P, 1], f32, name="sc2", tag="sc2")
                        nc.vector.tensor_scalar_mul(
                            sc2[:, :], inv[:, 4 + q_:5 + q_],
                            lam_b[:, h:h + 1])
                        tmp2 = cmb.tile([P, P], f32, name="tmp2", tag="tmp2")
                        nc.vector.tensor_scalar_mul(tmp2[:, :], u1, sc2[:, :])
                        attn_sl = cmb.tile([P, P], f16, name="attn_sl",
                                           tag="attn_sl")
                        nc.vector.scalar_tensor_tensor(
                            attn_sl[:, :], u0,
                            inv[:, q_:q_ + 1], tmp2[:, :],
                            mybir.AluOpType.mult, mybir.AluOpType.subtract)
                        ps_t = misc_ps.tile([P, P], f16, name="ps_at",
                                            tag="misc")
                        nc.tensor.transpose(ps_t[:, :], attn_sl[:, :],
                                            id_sb[:, :])
                        nc.vector.tensor_copy(attnT[h][:, js * P:(js + 1) * P],
                                              ps_t[:, :])
                # ---- Stage 3 (per head): AllToAll into sequence shards ----
                if sc == NSC - 1 and h == 0:
                    nc.gpsimd.dma_start(
                        out=bounce_in[0][:].rearrange(
                            "(d p f) -> p d f", d=NCORES, f=SROWS),
                        in_=attnT[0][:, :].rearrange(
                            "p (d f) -> p d f", f=SROWS))
                    nc.gpsimd.collective_compute(
                        "AllToAll", mybir.AluOpType.bypass, replica_groups=rg,
                        ins=[bounce_in[0][:]], outs=[bounce_out[0][:]])
                    # unpack head 0 right away (waits on the cc via deps,
                    # and the gpsimd queue is otherwise idle here) so the
                    # head-0 o_proj can start the moment stage 2 ends
                    nc.gpsimd.dma_start(
                        out=aTl[0][:, :].rearrange("p (d f) -> p d f",
                                                   d=NCORES),
                        in_=bounce_out[0][:].rearrange(
                            "(d p f) -> p d f", d=NCORES, f=SROWS))
                if sc == NSC - 1 and h == 1:
                    for i in range(2):
                        nc.gpsimd.dma_start(
                            out=bounce_in2[i][:].rearrange(
                                "(d p f) -> p d f", d=NCORES, f=P),
                            in_=attnT[1][:, :].rearrange(
                                "p (d f) -> p d f", f=SROWS)[:, :,
                                                            i * P:(i + 1) * P])
                        nc.gpsimd.collective_compute(
                            "AllToAll", mybir.AluOpType.bypass,
                            replica_groups=rg,
                            ins=[bounce_in2[i][:]], outs=[bounce_out2[i][:]])
                        nc.gpsimd.dma_start(
                            out=aTl[1][:, :].rearrange(
                                "p (d f) -> p d f",
                                d=NCORES)[:, :, i * P:(i + 1) * P],
                            in_=bounce_out2[i][:].rearrange(
                                "(d p f) -> p d f", d=NCORES, f=P))

        # ---- Stage 4: o_proj over the local 256 rows ----
        with tc.tile_pool(name="o_p", bufs=4) as o_pool, \
             tc.tile_pool(name="ps4", bufs=2, space="PSUM") as ps4:
            # head-0 sections only need the first AllToAll: run ALL of their
            # partial o_proj groups while the second exchange is in flight.
            o_es = {}
            for dc in range(4):
                for st_ in range(2):
                    ps_e = misc_ps.tile([P, SCW], f32, name="ps_e", tag="misc")
                    for i, ht in enumerate(range(0, H, 2)):
                        nc.tensor.matmul(
                            ps_e[:, :],
                            aTl[0][:, (ht // 2) * SROWS + st_ * P:
                                   (ht // 2) * SROWS + (st_ + 1) * P],
                            wos_l[dc][:, ht * SCW:(ht + 1) * SCW],
                            start=(i == 0), stop=(i == H // 2 - 1))
                    o_e = o_pool.tile([P, SCW], f32, name="o_e",
                                      tag=f"o_e{dc}{st_}", bufs=1)
                    nc.vector.tensor_copy(o_e[:, :], ps_e[:, :])
                    o_es[(dc, st_)] = o_e
            # keep the PE (HAM) warm across the second AllToAll's wait
            # window so the head-1 pass below starts at full clock; the
            # result is parked in DRAM and never read.
            warm_ps = misc_ps.tile([P, SCW], f32, name="warm", tag="misc")
            for w_ in range(6):
                nc.tensor.matmul(warm_ps[:, :], v16[:, 0:P],
                                 aTl[0][:, 0:SCW], start=True, stop=True)
            warm_sb = o_pool.tile([P, SCW], f32, name="warm_sb",
                                  tag="warm_sb", bufs=1)
            nc.vector.tensor_copy(warm_sb[:, :], warm_ps[:, :])
            warm_dram = dram.tile([P * SCW], f32, name="warm_dram")
            nc.gpsimd.dma_start(
                out=warm_dram[:].rearrange("(p f) -> p f", f=SCW),
                in_=warm_sb[:, :])
            for st_ in range(2):
                for dc in range(4):
                    ps_o = ps4.tile([P, SCW], f32, name="ps_o", tag="o")
                    for i, ht in enumerate(range(1, H, 2)):
                        nc.tensor.matmul(
                            ps_o[:, :],
                            aTl[1][:, (ht // 2) * SROWS + st_ * P:
                                   (ht // 2) * SROWS + (st_ + 1) * P],
                            wos_l[dc][:, ht * SCW:(ht + 1) * SCW],
                            start=(i == 0), stop=(i == H // 2 - 1))
                    o_sb = o_pool.tile([P, SCW], f32, name="o_sb", tag="o_sb")
                    nc.vector.tensor_add(o_sb[:, :], ps_o[:, :],
                                         o_es[(dc, st_)][:, :])
                    nc.sync.dma_start(
                        out=o_out[st_ * P:(st_ + 1) * P,
                                  dc * SCW:(dc + 1) * SCW],
                        in_=o_sb[:, :])

    nc.compile()
    return nc


def _prep(x, freqs_cos, freqs_sin, Wq, Wk, Wv, Wo, lambda_param):
    """Host-side sharding/layout prep. Returns per-core input maps."""
    x2 = np.asarray(x, np.float32).reshape(S, D)
    # pre-tile: xT[p, (sc, dt, f)] = x[512sc+f, 128dt+p] so each stage-1
    # dt-group DMA is one contiguous slice
    xT = np.ascontiguousarray(
        x2.reshape(NSC, SCW, DT, P).transpose(3, 0, 2, 1)
        .reshape(P, NSC * DT * SCW).astype(np.float16))
    cosT = np.asarray(freqs_cos, np.float32).T
    sinT = np.asarray(freqs_sin, np.float32).T
    cosT = np.ascontiguousarray(
        np.concatenate([cosT, cosT], axis=0).astype(np.float16))
    sinT = np.ascontiguousarray(
        np.concatenate([sinT, sinT], axis=0).astype(np.float16))
    Wq = np.asarray(Wq, np.float32)
    Wk = np.asarray(Wk, np.float32)
    Wv = np.asarray(Wv, np.float32)
    # pre-tile: wo[p, (dc, ht, f)] = Wo[128ht+p, 512dc+f]
    Wo16 = np.ascontiguousarray(
        np.asarray(Wo, np.float32).reshape(H, P, 4, SCW)
        .transpose(1, 2, 0, 3).reshape(P, 4 * H * SCW).astype(np.float16))
    lamp = np.asarray(lambda_param, np.float32)

    # de-interleave complex pairs: [x0 (freqs 0..63) | x1 (freqs 0..63)]
    perm = np.concatenate([
        2 * np.arange(64), 2 * np.arange(64) + 1]).astype(np.int64)

    # in-block causal triangle: tri[t, s] = t <= s
    t_rel = np.arange(P)[:, None]
    s_rel = np.arange(P)[None, :]
    tri = np.where(t_rel <= s_rel, 1.0, 0.0).astype(np.float16)

    ident = np.eye(P, dtype=np.float16)

    in_maps = []
    for c in range(NCORES):
        g = c // 2
        cols = []
        for h in (2 * c, 2 * c + 1):
            cols.append(Wq[:, h * HD:(h + 1) * HD][:, perm])
        cols.append(Wk[:, g * HD:(g + 1) * HD][:, perm])
        cols.append(Wv[:, g * HD:(g + 1) * HD])
        # pre-tile: wall[p, (dt, j)] = W[128dt+p, j]
        wall = np.ascontiguousarray(
            np.concatenate(cols, axis=1).reshape(DT, P, 4 * P)
            .transpose(1, 0, 2).reshape(P, DT * 4 * P).astype(np.float16))
        in_maps.append({
            "xT": xT,
            "wall": wall,
            "wo": Wo16,
            "cosT": cosT,
            "sinT": sinT,
            "tri": tri,
            "ident": ident,
            "lam": np.ascontiguousarray(
                lamp[2 * c:2 * c + 2].reshape(1, HPC)),
        })
    return in_maps


def _run(inputs, trace=False):
    if "nc" not in _CACHE:
        _CACHE["nc"] = _build()
    nc = _CACHE["nc"]
    in_maps = _prep(**inputs)
    res = run_bass_kernel_spmd(nc, in_maps, core_ids=list(range(NCORES)),
                               trace=trace)
    out = np.concatenate([res.results[c]["o_out"] for c in range(NCORES)],
                         axis=0)
    return out.reshape(1, S, D), res


def kernel(**inputs):
    out, _ = _run(inputs)
    return out

